# revision 46
# baseline (speedup 1.0000x reference)
"""MultiHeadAttention Trainium2 kernel.

Sharding: 8 cores = 2 batches x 4 head-groups (4 heads each).
Each core computes, for its batch b and heads [h0, h0+4):
  qT/kT [256, T] and v [T, 256] from xT @ w_qkv slices (channel-major),
  S^T = k q^T per head ([k, q] layout, causal folded into the mask on host),
  P = exp(S^T/sqrt(D)) * expmaskT (bf16), attention out O^T = [v|1]^T P
  (ones column gives the softmax denominators for free), O^T copied out of
  PSUM unnormalized, normalized by 1/sums (DMA-broadcast across
  partitions), then the partial output projection y_heads @ w_proj[rows].
  The host sums the 4 partial projections per batch.

Schedule: phase A (projections) is emitted in 4 n-blocks (512 tokens
each), interleaved with the attention pipeline — query-slice qs of the
attention only needs n-blocks 0..qs, so attention for qs=0 starts right
after the first n-block while n-blocks 1..3 still stream in. This keeps
the PE busy end-to-end (avoiding the K=4/8 DVFS down-clock that fires on
PE idle gaps) and hides the exp (ACT-engine) cost of the softmax under
projection matmuls. Partial output projections are injected per token
tile as soon as each query slice is normalized. All matmul operands are
bf16 (fp32 PSUM accumulate). The causal block-diagonal is trimmed
(shorter S/PV streams + smaller exp/mul) at 128-key granularity.

Engine balance: input DMA issue is spread across the sync/gpsimd/scalar
queues (scalar is idle until the first exp at ~14us). PSUM->SBUF
staging is split between DVE and ACT (pvo halves, proj ns-halves), the
es-multiply for head 1 of each group and the norm muls for heads 2-3
run on gpsimd, so no single engine's elementwise backlog stalls the PE.
"""

import sys

sys.path.insert(0, "/opt/trn_rl_repo")

import ml_dtypes
import numpy as np

import concourse.bass as bass
import concourse.mybir as mybir
import concourse.tile as tile
from concourse import bacc
from concourse.bass_utils import run_bass_kernel_spmd

B, T, C, H, D = 2, 2048, 1024, 16, 64
HPC = 4  # heads per core
NCORES = 8
KC = C // 128  # 8 contraction chunks for the projections
NT = T // 128  # 16 token tiles
NQ = T // 512  # 4 query slices
F32, BF16 = mybir.dt.float32, mybir.dt.bfloat16
AF = mybir.ActivationFunctionType
NEG = np.float32(-1.0e30)

# group list (qs, g): for query-slice qs, key-group g covers k-chunks
# [2g, 2g+2) of 128 keys each; causality keeps g < 2qs+2.
GROUPS = [(qs, g) for qs in range(NQ) for g in range(2 * qs + 2)]
GIDX = {qg: i for i, qg in enumerate(GROUPS)}
QS_FIRST = {qs: GIDX[(qs, 0)] for qs in range(NQ)}

_cache = {}


def _build():
    nc = bacc.Bacc("TRN2", target_bir_lowering=False, debug=False, num_devices=NCORES)
    xt_d = nc.dram_tensor("xt", [C, T], BF16, kind="ExternalInput")
    maskt_d = nc.dram_tensor("maskt", [T, T], BF16, kind="ExternalInput")
    wqkv_d = nc.dram_tensor("wqkv", [C, 3 * HPC * D], BF16, kind="ExternalInput")
    wproj_d = nc.dram_tensor("wproj", [HPC * D, C], BF16, kind="ExternalInput")
    out_d = nc.dram_tensor("out", [T, C], BF16, kind="ExternalOutput")

    with tile.TileContext(nc) as tc:
        with (
            tc.tile_pool(name="ps", bufs=1, space="PSUM") as ps,
            tc.tile_pool(name="apool", bufs=1) as apool,
            tc.tile_pool(name="mpool", bufs=1) as mpool,
            tc.tile_pool(name="espool", bufs=7) as espool,
            tc.tile_pool(name="xpool", bufs=7) as xpool,
            tc.tile_pool(name="spool", bufs=2) as spool,
            tc.tile_pool(name="stpool", bufs=4) as stpool,
            tc.tile_pool(name="wpool", bufs=1) as wpool,
            tc.tile_pool(name="dpool", bufs=2, space="DRAM") as dpool,
        ):
            xt_sb = wpool.tile([128, KC, T], BF16, tag="xt")
            wqkv_sb = wpool.tile([128, KC, 3 * HPC * D], BF16, tag="wqkv")
            wproj_sb = wpool.tile([128, 2, C], BF16, tag="wproj")
            mask_sb = mpool.tile([128, len(GROUPS), 2, 512], BF16, tag="mask")
            xt_v = xt_d.ap().rearrange("(c p) t -> p c t", p=128)
            wqkv_v = wqkv_d.ap().rearrange("(c p) n -> p c n", p=128)
            maskt_v = maskt_d.ap().rearrange("(c p) q -> p c q", p=128)  # [128,16,T]

            # ---- input DMA issue order. Descriptor issue costs ~630ns per
            # dma_start per engine queue; spread across sync + gpsimd +
            # scalar (ACT is free until the first exp ~14us), ordered to
            # match phase-A kc-major consumption of ablock 0, then ablocks
            # 1..3 (xt n-slices), with mask groups between.
            def dma_mask(qs, eng):
                g0 = QS_FIRST[qs]
                ng = 2 * qs + 2
                for a in range(0, ng, 2):
                    b = min(a + 2, ng)
                    eng.dma_start(
                        out=mask_sb[:, g0 + a : g0 + b, :, :],
                        in_=maskt_v[:, 2 * a : 2 * b, qs * 512 : (qs + 1) * 512],
                    )

            # sync: wqkv (kc-serial, first chunk split for a fast first
            # matmul), then mask qs0/qs1, xt n=3 slices, mask qs2/qs3.
            nc.sync.dma_start(out=wqkv_sb[:, 0, 0:512], in_=wqkv_v[:, 0, 0:512])
            nc.sync.dma_start(out=wqkv_sb[:, 0, 512:768], in_=wqkv_v[:, 0, 512:768])
            for kc in range(1, KC):
                nc.sync.dma_start(out=wqkv_sb[:, kc, :], in_=wqkv_v[:, kc, :])
            dma_mask(0, nc.sync)
            dma_mask(1, nc.sync)
            for kc in range(KC):
                nc.sync.dma_start(
                    out=xt_sb[:, kc, 1536:2048], in_=xt_v[:, kc, 1536:2048]
                )
            dma_mask(2, nc.sync)
            # gpsimd: xt n=0 slices (ablock 0), then n=2.
            for kc in range(KC):
                nc.gpsimd.dma_start(out=xt_sb[:, kc, 0:512], in_=xt_v[:, kc, 0:512])
            for kc in range(KC):
                nc.gpsimd.dma_start(
                    out=xt_sb[:, kc, 1024:1536], in_=xt_v[:, kc, 1024:1536]
                )
            # gpsimd: xt n=1 slices (ablock 1) + wproj; scalar stays clear
            # for the exp stream (it has no slack).
            for kc in range(KC):
                nc.gpsimd.dma_start(
                    out=xt_sb[:, kc, 512:1024], in_=xt_v[:, kc, 512:1024]
                )
            nc.gpsimd.dma_start(
                out=wproj_sb, in_=wproj_d.ap().rearrange("(m p) n -> p m n", p=128)
            )
            dma_mask(3, nc.sync)

            # ---- persistent SBUF tiles
            qt_tiles = [
                apool.tile([128, T], BF16, tag=f"qt{m}", name=f"qt{m}") for m in range(2)
            ]
            kt_tiles = [
                apool.tile([128, T], BF16, tag=f"kt{m}", name=f"kt{m}") for m in range(2)
            ]
            v_sb = apool.tile([128, NT, HPC * 65], BF16, tag="v")
            v_4d = v_sb.rearrange("p t (h e) -> p t h e", h=HPC)
            yt_tiles = [
                apool.tile([128, T], BF16, tag=f"yt{m}", name=f"yt{m}") for m in range(2)
            ]
            ones_t = spool.tile([128, NT * HPC], F32, tag="ones", bufs=1, name="ones_t")
            nc.vector.memset(ones_t, 1.0)
            ones_col = spool.tile([1, 64], BF16, tag="onec", bufs=1, name="ones_col")
            nc.vector.memset(ones_col, 1.0)
            nc.scalar.activation(
                v_4d[:, :, :, 64:65],
                ones_t.rearrange("p (t h one) -> p t h one", t=NT, one=1),
                AF.Copy,
            )

            # ---- phase A block for one n-slice: qt/kt columns + v token
            # tiles [4n, 4n+4). The v chains (pv banks) are emitted
            # immediately — right after the pvout that freed their psum —
            # while the four qt/kt accumulation chains are 1-bank sp-slot
            # pieces that can be woven between attention groups so the
            # exp/es pipeline stays fed during phase-A stretches.
            def emit_ablock(n, pieces=None):
                sl = slice(n * 512, (n + 1) * 512)
                # one psum BANK per v chain: interleaved open accumulation
                # chains must not share a bank (two chains at 256-offsets
                # inside one bank corrupt each other).
                v_ps = ps.tile([128, 4, 512], F32, tag="pv", bufs=1, name="v_ps")
                for kc in range(KC):
                    for i in range(4):
                        tt = 4 * n + i
                        nc.tensor.matmul(
                            v_ps[:, i, 0:256],
                            lhsT=xt_sb[:, kc, tt * 128 : (tt + 1) * 128],
                            rhs=wqkv_sb[:, kc, 2 * HPC * D : 3 * HPC * D],
                            start=(kc == 0),
                            stop=(kc == KC - 1),
                        )
                for i in range(4):
                    tt = 4 * n + i
                    nc.vector.tensor_copy(
                        v_4d[:, tt, :, 0:64],
                        v_ps[:, i, 0:256].rearrange("p (h d) -> p h d", h=HPC),
                    )

                def chain(base, dst, m):
                    def go():
                        acc = ps.tile([128, 512], F32, tag="sp", bufs=4, name="ab_ps")
                        for kc in range(KC):
                            nc.tensor.matmul(
                                acc,
                                lhsT=wqkv_sb[
                                    :, kc, base + m * 128 : base + (m + 1) * 128
                                ],
                                rhs=xt_sb[:, kc, sl],
                                start=(kc == 0),
                                stop=(kc == KC - 1),
                            )
                        nc.vector.tensor_copy(dst[:, sl], acc)

                    return go

                todo = [
                    chain(0, qt_tiles[0], 0),
                    chain(HPC * D, kt_tiles[0], 0),
                    chain(0, qt_tiles[1], 1),
                    chain(HPC * D, kt_tiles[1], 1),
                ]
                if pieces is None:
                    for p in todo:
                        p()
                else:
                    pieces.extend(todo)

            # ---- attention pieces
            def emit_s_group(qs, g):
                """S^T matmuls for group g (k-chunks 2g, 2g+1) of q-slice qs.
                On the causal diagonal the streams are trimmed: k-chunk c
                (relative to the diag) only serves queries q >= 128c."""
                sps = []
                for h in range(HPC):
                    mh, ph = divmod(h, 2)
                    p0 = ph * 64
                    pair = []
                    for i in range(2):
                        kc = 2 * g + i
                        c = kc - 4 * qs  # chunk position relative to diagonal
                        qoff = 128 * c if c > 0 else 0
                        sp = ps.tile([128, 512], F32, tag="sp", bufs=4, name="sp")
                        nc.tensor.matmul(
                            sp[:, qoff:512],
                            lhsT=kt_tiles[mh][p0 : p0 + 64, kc * 128 : (kc + 1) * 128],
                            rhs=qt_tiles[mh][p0 : p0 + 64, qs * 512 + qoff : (qs + 1) * 512],
                            start=True,
                            stop=True,
                        )
                        pair.append(sp)
                    sps.append(pair)
                return sps

            def emit_em(qs, g, sps):
                """P = exp(S) * expmask (bf16). For the second diagonal group
                (c=2,3) only the live region is processed; PV reads match.
                Head 1's multiply runs on gpsimd to offload the DVE."""
                gi = GIDX[(qs, g)]
                diag2 = g == 2 * qs + 1
                tiles = []
                for h in range(HPC):
                    exps = xpool.tile([128, 2, 512], BF16, tag="exps", name="exps")
                    es = espool.tile([128, 2, 512], BF16, tag="es", name="es")
                    if diag2:
                        nc.scalar.activation(
                            exps[:, 0, 256:512], sps[h][0][:, 256:512], AF.Exp
                        )
                        nc.scalar.activation(
                            exps[:, 1, 384:512], sps[h][1][:, 384:512], AF.Exp
                        )
                        nc.vector.tensor_mul(
                            es[:, :, 384:512],
                            exps[:, :, 384:512],
                            mask_sb[:, gi, :, 384:512],
                        )
                        nc.vector.tensor_mul(
                            es[:, 0, 256:384],
                            exps[:, 0, 256:384],
                            mask_sb[:, gi, 0, 256:384],
                        )
                    else:
                        for i in range(2):
                            nc.scalar.activation(exps[:, i, :], sps[h][i], AF.Exp)
                        nc.vector.tensor_mul(es, exps, mask_sb[:, gi, :, :])
                    tiles.append(es)
                return tiles

            pv_tiles = {}

            def emit_pv(qs, g, tiles):
                if g == 0:
                    pv_tiles[qs] = ps.tile(
                        [65, HPC, 512], F32, tag="pv", bufs=1, name="pv_all"
                    )
                nkc = 4 * qs + 4
                for h in range(HPC):
                    for i in range(2):
                        kc = 2 * g + i
                        c = kc - 4 * qs
                        qoff = 128 * c if c > 0 else 0
                        nc.tensor.matmul(
                            pv_tiles[qs][:, h, qoff:512],
                            lhsT=v_sb[:, kc, h * 65 : (h + 1) * 65],
                            rhs=tiles[h][:, i, qoff:512],
                            start=(kc == 0),
                            stop=(kc == nkc - 1),
                            skip_group_check=(qoff > 0),
                        )

            def emit_pvout(qs):
                """copy O^T (+ sums row) out of PSUM so the pv bank frees up
                for the next q-slice, then kick off the 1/sums pipeline.
                Mid-kernel slices (latency hidden): sums [1,2048] -> [128,16]
                spread -> reciprocal (bf16) -> DRAM -> [64,HPC,512]
                partition-stride-0 broadcast, all on gpsimd. The LAST slice's
                chain is fully exposed, and the scatter/broadcast DMAs are
                descriptor-bound (~2-3us each), so it instead computes the
                reciprocal straight off the PSUM sums row and broadcasts
                across the 64 d-partitions with a K=1 ones-column matmul —
                no DMA in the chain at all."""
                pvo = spool.tile([65, HPC, 512], BF16, tag="pvo", name="pvo")
                pv = pv_tiles.pop(qs)
                if qs == NQ - 1:
                    # single-partition DVE reciprocal is ~6ns/elem — keep the
                    # reciprocal wide. Spread the sums row across partitions
                    # with 4 small parallel DMAs on the idle HWDGE queues,
                    # 1/x wide, gather back to one row, then broadcast across
                    # the 64 d-partitions with K=1 ones-column matmuls.
                    nc.vector.tensor_copy(pvo[:, 0:2, :], pv[:, 0:2, :])
                    nc.scalar.activation(pvo[:, 2:4, :], pv[:, 2:4, :], AF.Copy)
                    # one hop each way (DMA fixed latency dominates the
                    # descriptor count, so fewer/bigger hops win): sums row
                    # -> [64, 32] wide, 1/x, -> back to one row.
                    spread = spool.tile([64, 32], BF16, tag="spread", name="spread")
                    rspread = spool.tile([64, 32], BF16, tag="rspread", name="rsp")
                    r_row = spool.tile([1, HPC, 512], BF16, tag="rrow", name="r_row")
                    nc.sync.dma_start(out=spread, in_=pvo[64:65, :, :])
                    with nc.allow_low_precision("bf16 1/sums; rel-err budget 2e-2"):
                        nc.vector.reciprocal(rspread, spread)
                    nc.scalar.dma_start(out=r_row, in_=rspread)
                    # the broadcast matmuls are deferred to norm time (the
                    # PE queue is in-order: emitting them here would make
                    # every later-emitted projection tile wait out the
                    # spread/gather DMA latency behind them).
                    return pvo, (r_row,)
                # split the evacuation DVE/ACT so the pv banks free in half
                # the time — the next slice's PV and the pending a-block's
                # v chains are both gated on them.
                nc.vector.tensor_copy(pvo[:, 0:2, :], pv[:, 0:2, :])
                nc.scalar.activation(pvo[:, 2:4, :], pv[:, 2:4, :], AF.Copy)
                spread = spool.tile([128, 16], BF16, tag="spread", name="spread")
                nc.gpsimd.dma_start(out=spread, in_=pvo[64:65, :, :])
                rspread = spool.tile([128, 16], BF16, tag="rspread", name="rspread")
                with nc.allow_low_precision("bf16 1/sums; rel-err budget is 2e-2"):
                    nc.vector.reciprocal(rspread, spread)
                d2 = dpool.tile([HPC * 512], BF16, tag="d2", name="d2")
                nc.gpsimd.dma_start(out=d2.rearrange("(p e) -> p e", p=128), in_=rspread)
                bcast = spool.tile([64, HPC, 512], BF16, tag="bcast", name="bcast")
                bsrc = bass.AP(
                    tensor=d2.tensor,
                    offset=d2.offset,
                    ap=[[0, 64], [512, HPC], [1, 512]],
                )
                nc.gpsimd.dma_start(out=bcast, in_=bsrc)
                return pvo, bcast

            def emit_norm(qs, pvo, bcast):
                if isinstance(bcast, tuple):
                    # last slice: broadcast 1/sums across the 64 d-partitions
                    # with K=1 ones-column matmuls into the pv banks the
                    # slice freed, then normalize in two column halves so
                    # the first projection tiles can start while the second
                    # half is still multiplying.
                    (r_row,) = bcast
                    bc = ps.tile([64, HPC, 512], F32, tag="pv", bufs=1, name="bc")
                    for h in range(HPC):
                        nc.tensor.matmul(
                            bc[:, h, :],
                            lhsT=ones_col,
                            rhs=r_row[:, h, :],
                            start=True,
                            stop=True,
                        )
                    for half in range(2):
                        cs = slice(half * 256, (half + 1) * 256)
                        q0 = qs * 512 + half * 256
                        for h in range(HPC):
                            mh, ph = divmod(h, 2)
                            nc.vector.tensor_mul(
                                yt_tiles[mh][ph * 64 : (ph + 1) * 64, q0 : q0 + 256],
                                pvo[0:64, h, cs],
                                bc[:, h, cs],
                            )
                    return
                for h in range(HPC):
                    mh, ph = divmod(h, 2)
                    nc.vector.tensor_mul(
                        yt_tiles[mh][ph * 64 : (ph + 1) * 64, qs * 512 : (qs + 1) * 512],
                        pvo[0:64, h, :],
                        bcast[:, h, :],
                    )

            odma = [0]

            def emit_proj_tt(tt, late):
                """partial projection for one token tile; one 2-bank psum
                slot per tile. ns-halves pipeline: the ns0 cast overlaps the
                ns1 matmuls. `late` tiles (emitted once the exp stream is
                done) use the ACT engine/queue for the second half and never
                touch gpsimd (its software-DGE end drain gates the finish)."""
                st = stpool.tile([128, C], BF16, tag="stage", name="st")
                queues = [nc.sync, nc.scalar] if late else [nc.sync, nc.gpsimd]
                for ns in range(2):
                    pj_ps = ps.tile([128, 512], F32, tag="sp", bufs=4, name="pj_ps")
                    for m in range(2):
                        nc.tensor.matmul(
                            pj_ps,
                            lhsT=yt_tiles[m][:, tt * 128 : (tt + 1) * 128],
                            rhs=wproj_sb[:, m, ns * 512 : (ns + 1) * 512],
                            start=(m == 0),
                            stop=(m == 1),
                        )
                    if ns == 1 and late:
                        nc.scalar.activation(st[:, 512:1024], pj_ps, AF.Copy)
                    else:
                        nc.vector.tensor_copy(
                            st[:, ns * 512 : (ns + 1) * 512], pj_ps
                        )
                    # the very last tiles split each half across both HWDGE
                    # queues so the final transfers drain before the end
                    # barrier instead of gating it.
                    nsplit = 2 if tt >= NT - 2 else 1
                    step = 512 // nsplit
                    for k in range(nsplit):
                        q = queues[odma[0] % len(queues)]
                        odma[0] += 1
                        c0 = ns * 512 + k * step
                        q.dma_start(
                            out=out_d.ap()[tt * 128 : (tt + 1) * 128, c0 : c0 + step],
                            in_=st[:, c0 : c0 + step],
                        )

            # ---- interleaved schedule.
            # Pipeline over groups: step i emits S(i+2), em(i+1), PV(i).
            # A-blocks injected at fixed steps (dependencies: S of qs needs
            # n-blocks <= qs). norm after PV(qs) completes; proj token tiles
            # spread over subsequent steps.
            n = len(GROUPS)
            em_out = {}

            def stage_s(i):
                if i < n:
                    em_out[i] = (GROUPS[i], emit_s_group(*GROUPS[i]))

            def stage_em(i):
                if 0 <= i < n:
                    (qs, g), sps = em_out[i]
                    em_out[i] = ((qs, g), emit_em(qs, g, sps))

            def stage_pv(i):
                if not (0 <= i < n):
                    return None
                (qs, g), tiles = em_out.pop(i)
                emit_pv(qs, g, tiles)
                if g == 2 * qs + 1:
                    return qs
                return None

            emit_ablock(0)
            stage_s(0)
            stage_s(1)
            stage_em(0)
            emit_ablock(1)
            # A-blocks inject right after the pvout that frees their psum
            # (steps 1 and 5 are where qs0/qs1 finish). Two proj token tiles
            # per q-slice run promptly; two are deferred to the tail (steps
            # >= n) to keep the PE fed while the last norm chain drains.
            ablock_at = {1: 2, 5: 3}
            norm_q, proj_q = [], []
            # deferred proj tiles land after the last PV so the PE has
            # filler while the final 1/sums chain drains.
            tail_slots = [18, 19, 20, 21, 22, 23]
            pieces = []

            def drain(k=1):
                for _ in range(k):
                    if pieces:
                        pieces.pop(0)()

            for i in range(n + 10):
                while norm_q and norm_q[0][0] <= i:
                    _, dq, pvo, bcast = norm_q.pop(0)
                    emit_norm(dq, pvo, bcast)
                    if dq < NQ - 1:
                        proj_q.append((i + 1, 4 * dq))
                        proj_q.append((i + 2, 4 * dq + 1))
                        for k in (2, 3):
                            proj_q.append((tail_slots.pop(0), 4 * dq + k))
                    else:
                        for k in range(4):
                            proj_q.append((24 + k, 4 * dq + k))
                    proj_q.sort()
                fin = stage_pv(i)
                if fin is not None:
                    pvo, bcast = emit_pvout(fin)
                    # the last slice's norm waits 2 extra steps so the
                    # deferred projection tiles' vector casts aren't queued
                    # behind norm TTs that are waiting on the 1/sums chain.
                    norm_q.append((i + (3 if fin == NQ - 1 else 1), fin, pvo, bcast))
                while proj_q and proj_q[0][0] <= i:
                    slot, tt = proj_q.pop(0)
                    emit_proj_tt(tt, slot >= 18)
                if i in ablock_at:
                    emit_ablock(ablock_at[i], pieces)
                drain()
                stage_s(i + 2)
                drain()
                stage_em(i + 1)
                drain()
            while norm_q:
                _, dq, pvo, bcast = norm_q.pop(0)
                emit_norm(dq, pvo, bcast)
                for k in range(4):
                    proj_q.append((0, 4 * dq + k))
            while proj_q:
                _, tt = proj_q.pop(0)
                emit_proj_tt(tt, True)

    nc.compile()
    return nc


def _get_program():
    if "nc" not in _cache:
        _cache["nc"] = _build()
    return _cache["nc"]


def _prep_in_maps(x, mask, w_qkv, w_proj, head_mask):
    x = np.asarray(x, dtype=np.float32)
    mask = np.asarray(mask, dtype=np.float32)
    w_qkv = np.asarray(w_qkv, dtype=np.float32)
    w_proj = np.asarray(w_proj, dtype=np.float32)
    head_mask = np.asarray(head_mask, dtype=np.float32)

    idx = np.arange(T)
    causal_pen = np.where(idx[:, None] > idx[None, :], NEG, np.float32(0.0))  # [k, q]

    xts, maskts = [], []
    for b in range(B):
        xts.append(np.ascontiguousarray(x[b].T).astype(ml_dtypes.bfloat16))
        em = np.exp(np.ascontiguousarray(mask[b, 0].T) + causal_pen)
        maskts.append(em.astype(ml_dtypes.bfloat16))

    in_maps = []
    for core in range(NCORES):
        b, hg = divmod(core, NCORES // B)
        h0 = hg * HPC
        wq = w_qkv[:, h0 * D : (h0 + HPC) * D] * np.float32(0.125)  # 1/sqrt(D)
        wk = w_qkv[:, C + h0 * D : C + (h0 + HPC) * D]
        wv = w_qkv[:, 2 * C + h0 * D : 2 * C + (h0 + HPC) * D]
        wqkv_c = np.ascontiguousarray(np.concatenate([wq, wk, wv], axis=1)).astype(
            ml_dtypes.bfloat16
        )
        wp = w_proj[h0 * D : (h0 + HPC) * D, :] * np.repeat(head_mask[h0 : h0 + HPC], D)[:, None]
        in_maps.append(
            {
                "xt": xts[b],
                "maskt": maskts[b],
                "wqkv": wqkv_c,
                "wproj": np.ascontiguousarray(wp.astype(ml_dtypes.bfloat16)),
            }
        )
    return in_maps


def run(inputs, trace=False, trace_cores=None):
    nc = _get_program()
    in_maps = _prep_in_maps(**inputs)
    res = run_bass_kernel_spmd(
        nc,
        in_maps,
        list(range(NCORES)),
        trace=trace,
        trace_cores=trace_cores,
    )
    out = np.zeros((B, T, C), dtype=np.float32)
    for core in range(NCORES):
        out[core // (NCORES // B)] += np.asarray(
            res.results[core]["out"], dtype=np.float32
        )
    return out, res


def kernel(x, mask, w_qkv, w_proj, head_mask):
    out, _ = run(dict(x=x, mask=mask, w_qkv=w_qkv, w_proj=w_proj, head_mask=head_mask))
    return out


# revision 48
# speedup vs baseline: 1.0060x; 1.0060x over previous
"""MultiHeadAttention Trainium2 kernel.

Sharding: 8 cores = 2 batches x 4 head-groups (4 heads each).
Each core computes, for its batch b and heads [h0, h0+4):
  qT/kT [256, T] and v [T, 256] from xT @ w_qkv slices (channel-major),
  S^T = k q^T per head ([k, q] layout, causal folded into the mask on host),
  P = exp(S^T/sqrt(D)) * expmaskT (bf16), attention out O^T = [v|1]^T P
  (ones column gives the softmax denominators for free), O^T copied out of
  PSUM unnormalized, normalized by 1/sums (DMA-broadcast across
  partitions), then the partial output projection y_heads @ w_proj[rows].
  The host sums the 4 partial projections per batch.

Schedule: phase A (projections) is emitted in 4 n-blocks (512 tokens
each), interleaved with the attention pipeline — query-slice qs of the
attention only needs n-blocks 0..qs, so attention for qs=0 starts right
after the first n-block while n-blocks 1..3 still stream in. This keeps
the PE busy end-to-end (avoiding the K=4/8 DVFS down-clock that fires on
PE idle gaps) and hides the exp (ACT-engine) cost of the softmax under
projection matmuls. Partial output projections are injected per token
tile as soon as each query slice is normalized. All matmul operands are
bf16 (fp32 PSUM accumulate). The causal block-diagonal is trimmed
(shorter S/PV streams + smaller exp/mul) at 128-key granularity.

Engine balance: input DMA issue is spread across the sync/gpsimd/scalar
queues (scalar is idle until the first exp at ~14us). PSUM->SBUF
staging is split between DVE and ACT (pvo halves, proj ns-halves), the
es-multiply for head 1 of each group and the norm muls for heads 2-3
run on gpsimd, so no single engine's elementwise backlog stalls the PE.
"""

import sys

sys.path.insert(0, "/opt/trn_rl_repo")

import ml_dtypes
import numpy as np

import concourse.bass as bass
import concourse.mybir as mybir
import concourse.tile as tile
from concourse import bacc
from concourse.bass_utils import run_bass_kernel_spmd

B, T, C, H, D = 2, 2048, 1024, 16, 64
HPC = 4  # heads per core
NCORES = 8
KC = C // 128  # 8 contraction chunks for the projections
NT = T // 128  # 16 token tiles
NQ = T // 512  # 4 query slices
F32, BF16 = mybir.dt.float32, mybir.dt.bfloat16
AF = mybir.ActivationFunctionType
NEG = np.float32(-1.0e30)

# group list (qs, g): for query-slice qs, key-group g covers k-chunks
# [2g, 2g+2) of 128 keys each; causality keeps g < 2qs+2.
GROUPS = [(qs, g) for qs in range(NQ) for g in range(2 * qs + 2)]
GIDX = {qg: i for i, qg in enumerate(GROUPS)}
QS_FIRST = {qs: GIDX[(qs, 0)] for qs in range(NQ)}

_cache = {}


def _build():
    nc = bacc.Bacc("TRN2", target_bir_lowering=False, debug=False, num_devices=NCORES)
    xt_d = nc.dram_tensor("xt", [C, T], BF16, kind="ExternalInput")
    maskt_d = nc.dram_tensor("maskt", [T, T], BF16, kind="ExternalInput")
    wqkv_d = nc.dram_tensor("wqkv", [C, 3 * HPC * D], BF16, kind="ExternalInput")
    wproj_d = nc.dram_tensor("wproj", [HPC * D, C], BF16, kind="ExternalInput")
    out_d = nc.dram_tensor("out", [T, C], BF16, kind="ExternalOutput")

    with tile.TileContext(nc) as tc:
        with (
            tc.tile_pool(name="ps", bufs=1, space="PSUM") as ps,
            tc.tile_pool(name="apool", bufs=1) as apool,
            tc.tile_pool(name="mpool", bufs=1) as mpool,
            tc.tile_pool(name="espool", bufs=7) as espool,
            tc.tile_pool(name="xpool", bufs=7) as xpool,
            tc.tile_pool(name="spool", bufs=2) as spool,
            tc.tile_pool(name="stpool", bufs=4) as stpool,
            tc.tile_pool(name="wpool", bufs=1) as wpool,
            tc.tile_pool(name="dpool", bufs=2, space="DRAM") as dpool,
        ):
            xt_sb = wpool.tile([128, KC, T], BF16, tag="xt")
            wqkv_sb = wpool.tile([128, KC, 3 * HPC * D], BF16, tag="wqkv")
            wproj_sb = wpool.tile([128, 2, C], BF16, tag="wproj")
            mask_sb = mpool.tile([128, len(GROUPS), 2, 512], BF16, tag="mask")
            xt_v = xt_d.ap().rearrange("(c p) t -> p c t", p=128)
            wqkv_v = wqkv_d.ap().rearrange("(c p) n -> p c n", p=128)
            maskt_v = maskt_d.ap().rearrange("(c p) q -> p c q", p=128)  # [128,16,T]

            # ---- input DMA issue order. Descriptor issue costs ~630ns per
            # dma_start per engine queue; spread across sync + gpsimd +
            # scalar (ACT is free until the first exp ~14us), ordered to
            # match phase-A kc-major consumption of ablock 0, then ablocks
            # 1..3 (xt n-slices), with mask groups between.
            def dma_mask(qs, eng):
                g0 = QS_FIRST[qs]
                ng = 2 * qs + 2
                for a in range(0, ng, 2):
                    b = min(a + 2, ng)
                    eng.dma_start(
                        out=mask_sb[:, g0 + a : g0 + b, :, :],
                        in_=maskt_v[:, 2 * a : 2 * b, qs * 512 : (qs + 1) * 512],
                    )

            # sync: wqkv (kc-serial, first chunk split for a fast first
            # matmul), then mask qs0/qs1, xt n=3 slices, mask qs2/qs3.
            nc.sync.dma_start(out=wqkv_sb[:, 0, 0:512], in_=wqkv_v[:, 0, 0:512])
            nc.sync.dma_start(out=wqkv_sb[:, 0, 512:768], in_=wqkv_v[:, 0, 512:768])
            for kc in range(1, KC):
                nc.sync.dma_start(out=wqkv_sb[:, kc, :], in_=wqkv_v[:, kc, :])
            dma_mask(0, nc.sync)
            dma_mask(1, nc.sync)
            for kc in range(KC):
                nc.sync.dma_start(
                    out=xt_sb[:, kc, 1536:2048], in_=xt_v[:, kc, 1536:2048]
                )
            dma_mask(2, nc.sync)
            # gpsimd: xt n=0 slices (ablock 0), then n=2.
            for kc in range(KC):
                nc.gpsimd.dma_start(out=xt_sb[:, kc, 0:512], in_=xt_v[:, kc, 0:512])
            for kc in range(KC):
                nc.gpsimd.dma_start(
                    out=xt_sb[:, kc, 1024:1536], in_=xt_v[:, kc, 1024:1536]
                )
            # gpsimd: xt n=1 slices (ablock 1) + wproj; scalar stays clear
            # for the exp stream (it has no slack).
            for kc in range(KC):
                nc.gpsimd.dma_start(
                    out=xt_sb[:, kc, 512:1024], in_=xt_v[:, kc, 512:1024]
                )
            nc.gpsimd.dma_start(
                out=wproj_sb, in_=wproj_d.ap().rearrange("(m p) n -> p m n", p=128)
            )
            dma_mask(3, nc.sync)

            # ---- persistent SBUF tiles
            qt_tiles = [
                apool.tile([128, T], BF16, tag=f"qt{m}", name=f"qt{m}") for m in range(2)
            ]
            kt_tiles = [
                apool.tile([128, T], BF16, tag=f"kt{m}", name=f"kt{m}") for m in range(2)
            ]
            v_sb = apool.tile([128, NT, HPC * 65], BF16, tag="v")
            v_4d = v_sb.rearrange("p t (h e) -> p t h e", h=HPC)
            yt_tiles = [
                apool.tile([128, T], BF16, tag=f"yt{m}", name=f"yt{m}") for m in range(2)
            ]
            ones_t = spool.tile([128, NT * HPC], F32, tag="ones", bufs=1, name="ones_t")
            nc.vector.memset(ones_t, 1.0)
            ones_col = spool.tile([1, 64], BF16, tag="onec", bufs=1, name="ones_col")
            nc.vector.memset(ones_col, 1.0)
            nc.scalar.activation(
                v_4d[:, :, :, 64:65],
                ones_t.rearrange("p (t h one) -> p t h one", t=NT, one=1),
                AF.Copy,
            )

            # ---- phase A block for one n-slice: qt/kt columns + v token
            # tiles [4n, 4n+4). The v chains (pv banks) are emitted
            # immediately — right after the pvout that freed their psum —
            # while the four qt/kt accumulation chains are 1-bank sp-slot
            # pieces that can be woven between attention groups so the
            # exp/es pipeline stays fed during phase-A stretches.
            def emit_ablock(n, pieces=None):
                sl = slice(n * 512, (n + 1) * 512)
                # one psum BANK per v chain: interleaved open accumulation
                # chains must not share a bank (two chains at 256-offsets
                # inside one bank corrupt each other).
                v_ps = ps.tile([128, 4, 512], F32, tag="pv", bufs=1, name="v_ps")
                for kc in range(KC):
                    for i in range(4):
                        tt = 4 * n + i
                        nc.tensor.matmul(
                            v_ps[:, i, 0:256],
                            lhsT=xt_sb[:, kc, tt * 128 : (tt + 1) * 128],
                            rhs=wqkv_sb[:, kc, 2 * HPC * D : 3 * HPC * D],
                            start=(kc == 0),
                            stop=(kc == KC - 1),
                        )
                for i in range(4):
                    tt = 4 * n + i
                    nc.vector.tensor_copy(
                        v_4d[:, tt, :, 0:64],
                        v_ps[:, i, 0:256].rearrange("p (h d) -> p h d", h=HPC),
                    )

                def chain(base, dst, m):
                    def go():
                        acc = ps.tile([128, 512], F32, tag="sp", bufs=4, name="ab_ps")
                        for kc in range(KC):
                            nc.tensor.matmul(
                                acc,
                                lhsT=wqkv_sb[
                                    :, kc, base + m * 128 : base + (m + 1) * 128
                                ],
                                rhs=xt_sb[:, kc, sl],
                                start=(kc == 0),
                                stop=(kc == KC - 1),
                            )
                        if n <= 1:
                            # the first two a-blocks run before the exp
                            # stream ramps up — ACT has slack there and the
                            # DVE is the early bottleneck.
                            nc.scalar.activation(dst[:, sl], acc, AF.Copy)
                        else:
                            nc.vector.tensor_copy(dst[:, sl], acc)

                    return go

                todo = [
                    chain(0, qt_tiles[0], 0),
                    chain(HPC * D, kt_tiles[0], 0),
                    chain(0, qt_tiles[1], 1),
                    chain(HPC * D, kt_tiles[1], 1),
                ]
                if pieces is None:
                    for p in todo:
                        p()
                else:
                    pieces.extend(todo)

            # ---- attention pieces
            def emit_s_group(qs, g):
                """S^T matmuls for group g (k-chunks 2g, 2g+1) of q-slice qs.
                On the causal diagonal the streams are trimmed: k-chunk c
                (relative to the diag) only serves queries q >= 128c."""
                sps = []
                for h in range(HPC):
                    mh, ph = divmod(h, 2)
                    p0 = ph * 64
                    pair = []
                    for i in range(2):
                        kc = 2 * g + i
                        c = kc - 4 * qs  # chunk position relative to diagonal
                        qoff = 128 * c if c > 0 else 0
                        sp = ps.tile([128, 512], F32, tag="sp", bufs=4, name="sp")
                        nc.tensor.matmul(
                            sp[:, qoff:512],
                            lhsT=kt_tiles[mh][p0 : p0 + 64, kc * 128 : (kc + 1) * 128],
                            rhs=qt_tiles[mh][p0 : p0 + 64, qs * 512 + qoff : (qs + 1) * 512],
                            start=True,
                            stop=True,
                        )
                        pair.append(sp)
                    sps.append(pair)
                return sps

            def emit_em(qs, g, sps):
                """P = exp(S) * expmask (bf16). For the second diagonal group
                (c=2,3) only the live region is processed; PV reads match.
                Head 1's multiply runs on gpsimd to offload the DVE."""
                gi = GIDX[(qs, g)]
                diag2 = g == 2 * qs + 1
                tiles = []
                for h in range(HPC):
                    exps = xpool.tile([128, 2, 512], BF16, tag="exps", name="exps")
                    es = espool.tile([128, 2, 512], BF16, tag="es", name="es")
                    if diag2:
                        nc.scalar.activation(
                            exps[:, 0, 256:512], sps[h][0][:, 256:512], AF.Exp
                        )
                        nc.scalar.activation(
                            exps[:, 1, 384:512], sps[h][1][:, 384:512], AF.Exp
                        )
                        nc.vector.tensor_mul(
                            es[:, :, 384:512],
                            exps[:, :, 384:512],
                            mask_sb[:, gi, :, 384:512],
                        )
                        nc.vector.tensor_mul(
                            es[:, 0, 256:384],
                            exps[:, 0, 256:384],
                            mask_sb[:, gi, 0, 256:384],
                        )
                    else:
                        for i in range(2):
                            nc.scalar.activation(exps[:, i, :], sps[h][i], AF.Exp)
                        nc.vector.tensor_mul(es, exps, mask_sb[:, gi, :, :])
                    tiles.append(es)
                return tiles

            pv_tiles = {}

            def emit_pv(qs, g, tiles):
                if g == 0:
                    pv_tiles[qs] = ps.tile(
                        [65, HPC, 512], F32, tag="pv", bufs=1, name="pv_all"
                    )
                nkc = 4 * qs + 4
                for h in range(HPC):
                    for i in range(2):
                        kc = 2 * g + i
                        c = kc - 4 * qs
                        qoff = 128 * c if c > 0 else 0
                        nc.tensor.matmul(
                            pv_tiles[qs][:, h, qoff:512],
                            lhsT=v_sb[:, kc, h * 65 : (h + 1) * 65],
                            rhs=tiles[h][:, i, qoff:512],
                            start=(kc == 0),
                            stop=(kc == nkc - 1),
                            skip_group_check=(qoff > 0),
                        )

            def emit_pvout(qs):
                """copy O^T (+ sums row) out of PSUM so the pv bank frees up
                for the next q-slice, then kick off the 1/sums pipeline.
                Mid-kernel slices (latency hidden): sums [1,2048] -> [128,16]
                spread -> reciprocal (bf16) -> DRAM -> [64,HPC,512]
                partition-stride-0 broadcast, all on gpsimd. The LAST slice's
                chain is fully exposed, and the scatter/broadcast DMAs are
                descriptor-bound (~2-3us each), so it instead computes the
                reciprocal straight off the PSUM sums row and broadcasts
                across the 64 d-partitions with a K=1 ones-column matmul —
                no DMA in the chain at all."""
                pvo = spool.tile([65, HPC, 512], BF16, tag="pvo", name="pvo")
                pv = pv_tiles.pop(qs)
                if qs == NQ - 1:
                    # single-partition DVE reciprocal is ~6ns/elem — keep the
                    # reciprocal wide. Spread the sums row across partitions
                    # with 4 small parallel DMAs on the idle HWDGE queues,
                    # 1/x wide, gather back to one row, then broadcast across
                    # the 64 d-partitions with K=1 ones-column matmuls.
                    nc.vector.tensor_copy(pvo[:, 0:2, :], pv[:, 0:2, :])
                    nc.scalar.activation(pvo[:, 2:4, :], pv[:, 2:4, :], AF.Copy)
                    # one hop each way (DMA fixed latency dominates the
                    # descriptor count, so fewer/bigger hops win): sums row
                    # -> [64, 32] wide, 1/x, -> back to one row.
                    spread = spool.tile([64, 32], BF16, tag="spread", name="spread")
                    rspread = spool.tile([64, 32], BF16, tag="rspread", name="rsp")
                    r_row = spool.tile([1, HPC, 512], BF16, tag="rrow", name="r_row")
                    nc.sync.dma_start(out=spread, in_=pvo[64:65, :, :])
                    with nc.allow_low_precision("bf16 1/sums; rel-err budget 2e-2"):
                        nc.vector.reciprocal(rspread, spread)
                    nc.scalar.dma_start(out=r_row, in_=rspread)
                    # the broadcast matmuls are deferred to norm time (the
                    # PE queue is in-order: emitting them here would make
                    # every later-emitted projection tile wait out the
                    # spread/gather DMA latency behind them).
                    return pvo, (r_row,)
                nc.vector.tensor_copy(pvo, pv)
                spread = spool.tile([128, 16], BF16, tag="spread", name="spread")
                nc.gpsimd.dma_start(out=spread, in_=pvo[64:65, :, :])
                rspread = spool.tile([128, 16], BF16, tag="rspread", name="rspread")
                with nc.allow_low_precision("bf16 1/sums; rel-err budget is 2e-2"):
                    nc.vector.reciprocal(rspread, spread)
                d2 = dpool.tile([HPC * 512], BF16, tag="d2", name="d2")
                nc.gpsimd.dma_start(out=d2.rearrange("(p e) -> p e", p=128), in_=rspread)
                bcast = spool.tile([64, HPC, 512], BF16, tag="bcast", name="bcast")
                bsrc = bass.AP(
                    tensor=d2.tensor,
                    offset=d2.offset,
                    ap=[[0, 64], [512, HPC], [1, 512]],
                )
                nc.gpsimd.dma_start(out=bcast, in_=bsrc)
                return pvo, bcast

            def emit_norm(qs, pvo, bcast):
                if isinstance(bcast, tuple):
                    # last slice: broadcast 1/sums across the 64 d-partitions
                    # with K=1 ones-column matmuls into the pv banks the
                    # slice freed, then normalize in two column halves so
                    # the first projection tiles can start while the second
                    # half is still multiplying.
                    (r_row,) = bcast
                    bc = ps.tile([64, HPC, 512], F32, tag="pv", bufs=1, name="bc")
                    for h in range(HPC):
                        nc.tensor.matmul(
                            bc[:, h, :],
                            lhsT=ones_col,
                            rhs=r_row[:, h, :],
                            start=True,
                            stop=True,
                        )
                    for half in range(2):
                        cs = slice(half * 256, (half + 1) * 256)
                        q0 = qs * 512 + half * 256
                        for h in range(HPC):
                            mh, ph = divmod(h, 2)
                            nc.vector.tensor_mul(
                                yt_tiles[mh][ph * 64 : (ph + 1) * 64, q0 : q0 + 256],
                                pvo[0:64, h, cs],
                                bc[:, h, cs],
                            )
                    return
                for h in range(HPC):
                    mh, ph = divmod(h, 2)
                    nc.vector.tensor_mul(
                        yt_tiles[mh][ph * 64 : (ph + 1) * 64, qs * 512 : (qs + 1) * 512],
                        pvo[0:64, h, :],
                        bcast[:, h, :],
                    )

            odma = [0]

            def emit_proj_tt(tt, late):
                """partial projection for one token tile; one 2-bank psum
                slot per tile. ns-halves pipeline: the ns0 cast overlaps the
                ns1 matmuls. `late` tiles (emitted once the exp stream is
                done) use the ACT engine/queue for the second half and never
                touch gpsimd (its software-DGE end drain gates the finish)."""
                st = stpool.tile([128, C], BF16, tag="stage", name="st")
                queues = [nc.sync, nc.scalar] if late else [nc.sync, nc.gpsimd]
                for ns in range(2):
                    pj_ps = ps.tile([128, 512], F32, tag="sp", bufs=4, name="pj_ps")
                    for m in range(2):
                        nc.tensor.matmul(
                            pj_ps,
                            lhsT=yt_tiles[m][:, tt * 128 : (tt + 1) * 128],
                            rhs=wproj_sb[:, m, ns * 512 : (ns + 1) * 512],
                            start=(m == 0),
                            stop=(m == 1),
                        )
                    if ns == 1 and late:
                        nc.scalar.activation(st[:, 512:1024], pj_ps, AF.Copy)
                    else:
                        nc.vector.tensor_copy(
                            st[:, ns * 512 : (ns + 1) * 512], pj_ps
                        )
                    # the very last tiles split each half across both HWDGE
                    # queues so the final transfers drain before the end
                    # barrier instead of gating it.
                    nsplit = 2 if tt >= NT - 2 else 1
                    step = 512 // nsplit
                    for k in range(nsplit):
                        q = queues[odma[0] % len(queues)]
                        odma[0] += 1
                        c0 = ns * 512 + k * step
                        q.dma_start(
                            out=out_d.ap()[tt * 128 : (tt + 1) * 128, c0 : c0 + step],
                            in_=st[:, c0 : c0 + step],
                        )

            # ---- interleaved schedule.
            # Pipeline over groups: step i emits S(i+2), em(i+1), PV(i).
            # A-blocks injected at fixed steps (dependencies: S of qs needs
            # n-blocks <= qs). norm after PV(qs) completes; proj token tiles
            # spread over subsequent steps.
            n = len(GROUPS)
            em_out = {}

            def stage_s(i):
                if i < n:
                    em_out[i] = (GROUPS[i], emit_s_group(*GROUPS[i]))

            def stage_em(i):
                if 0 <= i < n:
                    (qs, g), sps = em_out[i]
                    em_out[i] = ((qs, g), emit_em(qs, g, sps))

            def stage_pv(i):
                if not (0 <= i < n):
                    return None
                (qs, g), tiles = em_out.pop(i)
                emit_pv(qs, g, tiles)
                if g == 2 * qs + 1:
                    return qs
                return None

            emit_ablock(0)
            stage_s(0)
            stage_s(1)
            stage_em(0)
            emit_ablock(1)
            # A-blocks inject right after the pvout that frees their psum
            # (steps 1 and 5 are where qs0/qs1 finish). Two proj token tiles
            # per q-slice run promptly; two are deferred to the tail (steps
            # >= n) to keep the PE fed while the last norm chain drains.
            ablock_at = {1: 2, 5: 3}
            norm_q, proj_q = [], []
            # deferred proj tiles land after the last PV so the PE has
            # filler while the final 1/sums chain drains.
            tail_slots = [18, 19, 20, 21, 22, 23]
            pieces = []

            def drain(k=1):
                for _ in range(k):
                    if pieces:
                        pieces.pop(0)()

            for i in range(n + 10):
                while norm_q and norm_q[0][0] <= i:
                    _, dq, pvo, bcast = norm_q.pop(0)
                    emit_norm(dq, pvo, bcast)
                    if dq < NQ - 1:
                        proj_q.append((i + 1, 4 * dq))
                        proj_q.append((i + 2, 4 * dq + 1))
                        for k in (2, 3):
                            proj_q.append((tail_slots.pop(0), 4 * dq + k))
                    else:
                        for k in range(4):
                            proj_q.append((24 + k, 4 * dq + k))
                    proj_q.sort()
                fin = stage_pv(i)
                if fin is not None:
                    pvo, bcast = emit_pvout(fin)
                    # the last slice's norm waits 2 extra steps so the
                    # deferred projection tiles' vector casts aren't queued
                    # behind norm TTs that are waiting on the 1/sums chain.
                    norm_q.append((i + (3 if fin == NQ - 1 else 1), fin, pvo, bcast))
                while proj_q and proj_q[0][0] <= i:
                    slot, tt = proj_q.pop(0)
                    emit_proj_tt(tt, slot >= 18)
                if i in ablock_at:
                    emit_ablock(ablock_at[i], pieces)
                drain()
                stage_s(i + 2)
                drain()
                stage_em(i + 1)
                drain()
            while norm_q:
                _, dq, pvo, bcast = norm_q.pop(0)
                emit_norm(dq, pvo, bcast)
                for k in range(4):
                    proj_q.append((0, 4 * dq + k))
            while proj_q:
                _, tt = proj_q.pop(0)
                emit_proj_tt(tt, True)

    nc.compile()
    return nc


def _get_program():
    if "nc" not in _cache:
        _cache["nc"] = _build()
    return _cache["nc"]


def _prep_in_maps(x, mask, w_qkv, w_proj, head_mask):
    x = np.asarray(x, dtype=np.float32)
    mask = np.asarray(mask, dtype=np.float32)
    w_qkv = np.asarray(w_qkv, dtype=np.float32)
    w_proj = np.asarray(w_proj, dtype=np.float32)
    head_mask = np.asarray(head_mask, dtype=np.float32)

    idx = np.arange(T)
    causal_pen = np.where(idx[:, None] > idx[None, :], NEG, np.float32(0.0))  # [k, q]

    xts, maskts = [], []
    for b in range(B):
        xts.append(np.ascontiguousarray(x[b].T).astype(ml_dtypes.bfloat16))
        em = np.exp(np.ascontiguousarray(mask[b, 0].T) + causal_pen)
        maskts.append(em.astype(ml_dtypes.bfloat16))

    in_maps = []
    for core in range(NCORES):
        b, hg = divmod(core, NCORES // B)
        h0 = hg * HPC
        wq = w_qkv[:, h0 * D : (h0 + HPC) * D] * np.float32(0.125)  # 1/sqrt(D)
        wk = w_qkv[:, C + h0 * D : C + (h0 + HPC) * D]
        wv = w_qkv[:, 2 * C + h0 * D : 2 * C + (h0 + HPC) * D]
        wqkv_c = np.ascontiguousarray(np.concatenate([wq, wk, wv], axis=1)).astype(
            ml_dtypes.bfloat16
        )
        wp = w_proj[h0 * D : (h0 + HPC) * D, :] * np.repeat(head_mask[h0 : h0 + HPC], D)[:, None]
        in_maps.append(
            {
                "xt": xts[b],
                "maskt": maskts[b],
                "wqkv": wqkv_c,
                "wproj": np.ascontiguousarray(wp.astype(ml_dtypes.bfloat16)),
            }
        )
    return in_maps


def run(inputs, trace=False, trace_cores=None):
    nc = _get_program()
    in_maps = _prep_in_maps(**inputs)
    res = run_bass_kernel_spmd(
        nc,
        in_maps,
        list(range(NCORES)),
        trace=trace,
        trace_cores=trace_cores,
    )
    out = np.zeros((B, T, C), dtype=np.float32)
    for core in range(NCORES):
        out[core // (NCORES // B)] += np.asarray(
            res.results[core]["out"], dtype=np.float32
        )
    return out, res


def kernel(x, mask, w_qkv, w_proj, head_mask):
    out, _ = run(dict(x=x, mask=mask, w_qkv=w_qkv, w_proj=w_proj, head_mask=head_mask))
    return out


# revision 49
# speedup vs baseline: 1.0173x; 1.0113x over previous
"""MultiHeadAttention Trainium2 kernel.

Sharding: 8 cores = 2 batches x 4 head-groups (4 heads each).
Each core computes, for its batch b and heads [h0, h0+4):
  qT/kT [256, T] and v [T, 256] from xT @ w_qkv slices (channel-major),
  S^T = k q^T per head ([k, q] layout, causal folded into the mask on host),
  P = exp(S^T/sqrt(D)) * expmaskT (bf16), attention out O^T = [v|1]^T P
  (ones column gives the softmax denominators for free), O^T copied out of
  PSUM unnormalized, normalized by 1/sums (DMA-broadcast across
  partitions), then the partial output projection y_heads @ w_proj[rows].
  The host sums the 4 partial projections per batch.

Schedule: phase A (projections) is emitted in 4 n-blocks (512 tokens
each), interleaved with the attention pipeline — query-slice qs of the
attention only needs n-blocks 0..qs, so attention for qs=0 starts right
after the first n-block while n-blocks 1..3 still stream in. This keeps
the PE busy end-to-end (avoiding the K=4/8 DVFS down-clock that fires on
PE idle gaps) and hides the exp (ACT-engine) cost of the softmax under
projection matmuls. Partial output projections are injected per token
tile as soon as each query slice is normalized. All matmul operands are
bf16 (fp32 PSUM accumulate). The causal block-diagonal is trimmed
(shorter S/PV streams + smaller exp/mul) at 128-key granularity.

Engine balance: input DMA issue is spread across the sync/gpsimd/scalar
queues (scalar is idle until the first exp at ~14us). PSUM->SBUF
staging is split between DVE and ACT (pvo halves, proj ns-halves), the
es-multiply for head 1 of each group and the norm muls for heads 2-3
run on gpsimd, so no single engine's elementwise backlog stalls the PE.
"""

import sys

sys.path.insert(0, "/opt/trn_rl_repo")

import ml_dtypes
import numpy as np

import concourse.bass as bass
import concourse.mybir as mybir
import concourse.tile as tile
from concourse import bacc
from concourse.bass_utils import run_bass_kernel_spmd

B, T, C, H, D = 2, 2048, 1024, 16, 64
HPC = 4  # heads per core
NCORES = 8
KC = C // 128  # 8 contraction chunks for the projections
NT = T // 128  # 16 token tiles
NQ = T // 512  # 4 query slices
F32, BF16 = mybir.dt.float32, mybir.dt.bfloat16
AF = mybir.ActivationFunctionType
NEG = np.float32(-1.0e30)

# group list (qs, g): for query-slice qs, key-group g covers k-chunks
# [2g, 2g+2) of 128 keys each; causality keeps g < 2qs+2.
GROUPS = [(qs, g) for qs in range(NQ) for g in range(2 * qs + 2)]
GIDX = {qg: i for i, qg in enumerate(GROUPS)}
QS_FIRST = {qs: GIDX[(qs, 0)] for qs in range(NQ)}

_cache = {}


def _build():
    nc = bacc.Bacc("TRN2", target_bir_lowering=False, debug=False, num_devices=NCORES)
    xt_d = nc.dram_tensor("xt", [C, T], BF16, kind="ExternalInput")
    maskt_d = nc.dram_tensor("maskt", [T, T], BF16, kind="ExternalInput")
    wqkv_d = nc.dram_tensor("wqkv", [C, 3 * HPC * D], BF16, kind="ExternalInput")
    wproj_d = nc.dram_tensor("wproj", [HPC * D, C], BF16, kind="ExternalInput")
    out_d = nc.dram_tensor("out", [T, C], BF16, kind="ExternalOutput")

    with tile.TileContext(nc) as tc:
        with (
            tc.tile_pool(name="ps", bufs=1, space="PSUM") as ps,
            tc.tile_pool(name="apool", bufs=1) as apool,
            tc.tile_pool(name="mpool", bufs=1) as mpool,
            tc.tile_pool(name="espool", bufs=7) as espool,
            tc.tile_pool(name="xpool", bufs=7) as xpool,
            tc.tile_pool(name="spool", bufs=2) as spool,
            tc.tile_pool(name="stpool", bufs=4) as stpool,
            tc.tile_pool(name="wpool", bufs=1) as wpool,
            tc.tile_pool(name="dpool", bufs=2, space="DRAM") as dpool,
        ):
            xt_sb = wpool.tile([128, KC, T], BF16, tag="xt")
            wqkv_sb = wpool.tile([128, KC, 3 * HPC * D], BF16, tag="wqkv")
            wproj_sb = wpool.tile([128, 2, C], BF16, tag="wproj")
            mask_sb = mpool.tile([128, len(GROUPS), 2, 512], BF16, tag="mask")
            xt_v = xt_d.ap().rearrange("(c p) t -> p c t", p=128)
            wqkv_v = wqkv_d.ap().rearrange("(c p) n -> p c n", p=128)
            maskt_v = maskt_d.ap().rearrange("(c p) q -> p c q", p=128)  # [128,16,T]

            # ---- input DMA issue order. Descriptor issue costs ~630ns per
            # dma_start per engine queue; spread across sync + gpsimd +
            # scalar (ACT is free until the first exp ~14us), ordered to
            # match phase-A kc-major consumption of ablock 0, then ablocks
            # 1..3 (xt n-slices), with mask groups between.
            def dma_mask(qs, eng):
                g0 = QS_FIRST[qs]
                ng = 2 * qs + 2
                for a in range(0, ng, 2):
                    b = min(a + 2, ng)
                    eng.dma_start(
                        out=mask_sb[:, g0 + a : g0 + b, :, :],
                        in_=maskt_v[:, 2 * a : 2 * b, qs * 512 : (qs + 1) * 512],
                    )

            # sync: wqkv (kc-serial, first chunk split for a fast first
            # matmul), then mask qs0/qs1, xt n=3 slices, mask qs2/qs3.
            nc.sync.dma_start(out=wqkv_sb[:, 0, 0:512], in_=wqkv_v[:, 0, 0:512])
            nc.sync.dma_start(out=wqkv_sb[:, 0, 512:768], in_=wqkv_v[:, 0, 512:768])
            for kc in range(1, KC):
                nc.sync.dma_start(out=wqkv_sb[:, kc, :], in_=wqkv_v[:, kc, :])
            dma_mask(0, nc.sync)
            dma_mask(1, nc.sync)
            for kc in range(KC):
                nc.sync.dma_start(
                    out=xt_sb[:, kc, 1536:2048], in_=xt_v[:, kc, 1536:2048]
                )
            dma_mask(2, nc.sync)
            # gpsimd: xt n=0 slices (ablock 0), then n=2.
            for kc in range(KC):
                nc.gpsimd.dma_start(out=xt_sb[:, kc, 0:512], in_=xt_v[:, kc, 0:512])
            for kc in range(KC):
                nc.gpsimd.dma_start(
                    out=xt_sb[:, kc, 1024:1536], in_=xt_v[:, kc, 1024:1536]
                )
            # gpsimd: xt n=1 slices (ablock 1) + wproj; scalar stays clear
            # for the exp stream (it has no slack).
            for kc in range(KC):
                nc.gpsimd.dma_start(
                    out=xt_sb[:, kc, 512:1024], in_=xt_v[:, kc, 512:1024]
                )
            nc.gpsimd.dma_start(
                out=wproj_sb, in_=wproj_d.ap().rearrange("(m p) n -> p m n", p=128)
            )
            dma_mask(3, nc.sync)

            # ---- persistent SBUF tiles
            qt_tiles = [
                apool.tile([128, T], BF16, tag=f"qt{m}", name=f"qt{m}") for m in range(2)
            ]
            kt_tiles = [
                apool.tile([128, T], BF16, tag=f"kt{m}", name=f"kt{m}") for m in range(2)
            ]
            v_sb = apool.tile([128, NT, HPC * 65], BF16, tag="v")
            v_4d = v_sb.rearrange("p t (h e) -> p t h e", h=HPC)
            yt_tiles = [
                apool.tile([128, T], BF16, tag=f"yt{m}", name=f"yt{m}") for m in range(2)
            ]
            ones_t = spool.tile([128, NT * HPC], F32, tag="ones", bufs=1, name="ones_t")
            nc.vector.memset(ones_t, 1.0)
            ones_col = spool.tile([1, 64], BF16, tag="onec", bufs=1, name="ones_col")
            nc.vector.memset(ones_col, 1.0)
            nc.scalar.activation(
                v_4d[:, :, :, 64:65],
                ones_t.rearrange("p (t h one) -> p t h one", t=NT, one=1),
                AF.Copy,
            )

            # ---- phase A block for one n-slice: qt/kt columns + v token
            # tiles [4n, 4n+4). The v chains (pv banks) are emitted
            # immediately — right after the pvout that freed their psum —
            # while the four qt/kt accumulation chains are 1-bank sp-slot
            # pieces that can be woven between attention groups so the
            # exp/es pipeline stays fed during phase-A stretches.
            def emit_ablock(n, pieces=None):
                sl = slice(n * 512, (n + 1) * 512)
                # one psum BANK per v chain: interleaved open accumulation
                # chains must not share a bank (two chains at 256-offsets
                # inside one bank corrupt each other).
                v_ps = ps.tile([128, 4, 512], F32, tag="pv", bufs=1, name="v_ps")
                for kc in range(KC):
                    for i in range(4):
                        tt = 4 * n + i
                        nc.tensor.matmul(
                            v_ps[:, i, 0:256],
                            lhsT=xt_sb[:, kc, tt * 128 : (tt + 1) * 128],
                            rhs=wqkv_sb[:, kc, 2 * HPC * D : 3 * HPC * D],
                            start=(kc == 0),
                            stop=(kc == KC - 1),
                        )
                for i in range(4):
                    tt = 4 * n + i
                    nc.vector.tensor_copy(
                        v_4d[:, tt, :, 0:64],
                        v_ps[:, i, 0:256].rearrange("p (h d) -> p h d", h=HPC),
                    )

                def chain(base, dst, m):
                    def go():
                        acc = ps.tile([128, 512], F32, tag="sp", bufs=4, name="ab_ps")
                        for kc in range(KC):
                            nc.tensor.matmul(
                                acc,
                                lhsT=wqkv_sb[
                                    :, kc, base + m * 128 : base + (m + 1) * 128
                                ],
                                rhs=xt_sb[:, kc, sl],
                                start=(kc == 0),
                                stop=(kc == KC - 1),
                            )
                        if n <= 1:
                            # the first two a-blocks run before the exp
                            # stream ramps up — ACT has slack there and the
                            # DVE is the early bottleneck.
                            nc.scalar.activation(dst[:, sl], acc, AF.Copy)
                        else:
                            nc.vector.tensor_copy(dst[:, sl], acc)

                    return go

                todo = [
                    chain(0, qt_tiles[0], 0),
                    chain(HPC * D, kt_tiles[0], 0),
                    chain(0, qt_tiles[1], 1),
                    chain(HPC * D, kt_tiles[1], 1),
                ]
                if pieces is None:
                    for p in todo:
                        p()
                else:
                    pieces.extend(todo)

            # ---- attention pieces
            def emit_s_group(qs, g):
                """S^T matmuls for group g (k-chunks 2g, 2g+1) of q-slice qs.
                On the causal diagonal the streams are trimmed: k-chunk c
                (relative to the diag) only serves queries q >= 128c."""
                sps = []
                for h in range(HPC):
                    mh, ph = divmod(h, 2)
                    p0 = ph * 64
                    pair = []
                    for i in range(2):
                        kc = 2 * g + i
                        c = kc - 4 * qs  # chunk position relative to diagonal
                        qoff = 128 * c if c > 0 else 0
                        sp = ps.tile([128, 512], F32, tag="sp", bufs=4, name="sp")
                        nc.tensor.matmul(
                            sp[:, qoff:512],
                            lhsT=kt_tiles[mh][p0 : p0 + 64, kc * 128 : (kc + 1) * 128],
                            rhs=qt_tiles[mh][p0 : p0 + 64, qs * 512 + qoff : (qs + 1) * 512],
                            start=True,
                            stop=True,
                        )
                        pair.append(sp)
                    sps.append(pair)
                return sps

            def emit_em(qs, g, sps):
                """P = exp(S) * expmask (bf16). For the second diagonal group
                (c=2,3) only the live region is processed; PV reads match.
                Head 1's multiply runs on gpsimd to offload the DVE."""
                gi = GIDX[(qs, g)]
                diag2 = g == 2 * qs + 1
                tiles = []
                for h in range(HPC):
                    exps = xpool.tile([128, 2, 512], BF16, tag="exps", name="exps")
                    es = espool.tile([128, 2, 512], BF16, tag="es", name="es")
                    if diag2:
                        nc.scalar.activation(
                            exps[:, 0, 256:512], sps[h][0][:, 256:512], AF.Exp
                        )
                        nc.scalar.activation(
                            exps[:, 1, 384:512], sps[h][1][:, 384:512], AF.Exp
                        )
                        nc.vector.tensor_mul(
                            es[:, :, 384:512],
                            exps[:, :, 384:512],
                            mask_sb[:, gi, :, 384:512],
                        )
                        nc.vector.tensor_mul(
                            es[:, 0, 256:384],
                            exps[:, 0, 256:384],
                            mask_sb[:, gi, 0, 256:384],
                        )
                    else:
                        for i in range(2):
                            nc.scalar.activation(exps[:, i, :], sps[h][i], AF.Exp)
                        nc.vector.tensor_mul(es, exps, mask_sb[:, gi, :, :])
                    tiles.append(es)
                return tiles

            pv_tiles = {}

            def emit_pv(qs, g, tiles):
                if g == 0:
                    pv_tiles[qs] = ps.tile(
                        [65, HPC, 512], F32, tag="pv", bufs=1, name="pv_all"
                    )
                nkc = 4 * qs + 4
                for h in range(HPC):
                    for i in range(2):
                        kc = 2 * g + i
                        c = kc - 4 * qs
                        qoff = 128 * c if c > 0 else 0
                        nc.tensor.matmul(
                            pv_tiles[qs][:, h, qoff:512],
                            lhsT=v_sb[:, kc, h * 65 : (h + 1) * 65],
                            rhs=tiles[h][:, i, qoff:512],
                            start=(kc == 0),
                            stop=(kc == nkc - 1),
                            skip_group_check=(qoff > 0),
                        )

            def emit_pvout(qs):
                """copy O^T (+ sums row) out of PSUM so the pv bank frees up
                for the next q-slice, then kick off the 1/sums pipeline.
                Mid-kernel slices (latency hidden): sums [1,2048] -> [128,16]
                spread -> reciprocal (bf16) -> DRAM -> [64,HPC,512]
                partition-stride-0 broadcast, all on gpsimd. The LAST slice's
                chain is fully exposed, and the scatter/broadcast DMAs are
                descriptor-bound (~2-3us each), so it instead computes the
                reciprocal straight off the PSUM sums row and broadcasts
                across the 64 d-partitions with a K=1 ones-column matmul —
                no DMA in the chain at all."""
                pvo = spool.tile([65, HPC, 512], BF16, tag="pvo", name="pvo")
                pv = pv_tiles.pop(qs)
                if qs == NQ - 1:
                    # single-partition DVE reciprocal is ~6ns/elem — keep the
                    # reciprocal wide. Spread the sums row across partitions
                    # with 4 small parallel DMAs on the idle HWDGE queues,
                    # 1/x wide, gather back to one row, then broadcast across
                    # the 64 d-partitions with K=1 ones-column matmuls.
                    nc.vector.tensor_copy(pvo[:, 0:2, :], pv[:, 0:2, :])
                    nc.scalar.activation(pvo[:, 2:4, :], pv[:, 2:4, :], AF.Copy)
                    # one hop each way (DMA fixed latency dominates the
                    # descriptor count, so fewer/bigger hops win): sums row
                    # -> [64, 32] wide, 1/x, -> back to one row.
                    spread = spool.tile([64, 32], BF16, tag="spread", name="spread")
                    rspread = spool.tile([64, 32], BF16, tag="rspread", name="rsp")
                    r_row = spool.tile([1, HPC, 512], BF16, tag="rrow", name="r_row")
                    nc.sync.dma_start(out=spread, in_=pvo[64:65, :, :])
                    with nc.allow_low_precision("bf16 1/sums; rel-err budget 2e-2"):
                        nc.vector.reciprocal(rspread, spread)
                    nc.scalar.dma_start(out=r_row, in_=rspread)
                    # the broadcast matmuls are deferred to norm time (the
                    # PE queue is in-order: emitting them here would make
                    # every later-emitted projection tile wait out the
                    # spread/gather DMA latency behind them).
                    return pvo, (r_row,)
                nc.vector.tensor_copy(pvo, pv)
                spread = spool.tile([128, 16], BF16, tag="spread", name="spread")
                nc.gpsimd.dma_start(out=spread, in_=pvo[64:65, :, :])
                rspread = spool.tile([128, 16], BF16, tag="rspread", name="rspread")
                with nc.allow_low_precision("bf16 1/sums; rel-err budget is 2e-2"):
                    nc.vector.reciprocal(rspread, spread)
                d2 = dpool.tile([HPC * 512], BF16, tag="d2", name="d2")
                nc.gpsimd.dma_start(out=d2.rearrange("(p e) -> p e", p=128), in_=rspread)
                bcast = spool.tile([64, HPC, 512], BF16, tag="bcast", name="bcast")
                bsrc = bass.AP(
                    tensor=d2.tensor,
                    offset=d2.offset,
                    ap=[[0, 64], [512, HPC], [1, 512]],
                )
                nc.gpsimd.dma_start(out=bcast, in_=bsrc)
                return pvo, bcast

            def emit_norm(qs, pvo, bcast):
                if isinstance(bcast, tuple):
                    # last slice: broadcast 1/sums across the 64 d-partitions
                    # with K=1 ones-column matmuls into the pv banks the
                    # slice freed, then normalize in two column halves so
                    # the first projection tiles can start while the second
                    # half is still multiplying.
                    (r_row,) = bcast
                    bc = ps.tile([64, HPC, 512], F32, tag="pv", bufs=1, name="bc")
                    for h in range(HPC):
                        nc.tensor.matmul(
                            bc[:, h, :],
                            lhsT=ones_col,
                            rhs=r_row[:, h, :],
                            start=True,
                            stop=True,
                        )
                    for half in range(2):
                        cs = slice(half * 256, (half + 1) * 256)
                        q0 = qs * 512 + half * 256
                        for h in range(HPC):
                            mh, ph = divmod(h, 2)
                            nc.vector.tensor_mul(
                                yt_tiles[mh][ph * 64 : (ph + 1) * 64, q0 : q0 + 256],
                                pvo[0:64, h, cs],
                                bc[:, h, cs],
                            )
                    return
                for h in range(HPC):
                    mh, ph = divmod(h, 2)
                    nc.vector.tensor_mul(
                        yt_tiles[mh][ph * 64 : (ph + 1) * 64, qs * 512 : (qs + 1) * 512],
                        pvo[0:64, h, :],
                        bcast[:, h, :],
                    )

            odma = [0]

            def emit_proj_tt(tt, late):
                """partial projection for one token tile; one 2-bank psum
                slot per tile. ns-halves pipeline: the ns0 cast overlaps the
                ns1 matmuls. `late` tiles (emitted once the exp stream is
                done) use the ACT engine/queue for the second half and never
                touch gpsimd (its software-DGE end drain gates the finish)."""
                st = stpool.tile([128, C], BF16, tag="stage", name="st")
                queues = [nc.sync, nc.scalar] if late else [nc.sync, nc.gpsimd]
                # m-major matmul order: consecutive instructions share the
                # stationary yt tile, halving weight reloads.
                pjs = [
                    ps.tile([128, 512], F32, tag="sp", bufs=4, name="pj_ps")
                    for _ in range(2)
                ]
                for m in range(2):
                    for ns in range(2):
                        nc.tensor.matmul(
                            pjs[ns],
                            lhsT=yt_tiles[m][:, tt * 128 : (tt + 1) * 128],
                            rhs=wproj_sb[:, m, ns * 512 : (ns + 1) * 512],
                            start=(m == 0),
                            stop=(m == 1),
                        )
                for ns in range(2):
                    pj_ps = pjs[ns]
                    if ns == 1 and late:
                        nc.scalar.activation(st[:, 512:1024], pj_ps, AF.Copy)
                    else:
                        nc.vector.tensor_copy(
                            st[:, ns * 512 : (ns + 1) * 512], pj_ps
                        )
                    # the very last tiles split each half across both HWDGE
                    # queues so the final transfers drain before the end
                    # barrier instead of gating it.
                    nsplit = 2 if tt >= NT - 2 else 1
                    step = 512 // nsplit
                    for k in range(nsplit):
                        q = queues[odma[0] % len(queues)]
                        odma[0] += 1
                        c0 = ns * 512 + k * step
                        q.dma_start(
                            out=out_d.ap()[tt * 128 : (tt + 1) * 128, c0 : c0 + step],
                            in_=st[:, c0 : c0 + step],
                        )

            # ---- interleaved schedule.
            # Pipeline over groups: step i emits S(i+2), em(i+1), PV(i).
            # A-blocks injected at fixed steps (dependencies: S of qs needs
            # n-blocks <= qs). norm after PV(qs) completes; proj token tiles
            # spread over subsequent steps.
            n = len(GROUPS)
            em_out = {}

            def stage_s(i):
                if i < n:
                    em_out[i] = (GROUPS[i], emit_s_group(*GROUPS[i]))

            def stage_em(i):
                if 0 <= i < n:
                    (qs, g), sps = em_out[i]
                    em_out[i] = ((qs, g), emit_em(qs, g, sps))

            def stage_pv(i):
                if not (0 <= i < n):
                    return None
                (qs, g), tiles = em_out.pop(i)
                emit_pv(qs, g, tiles)
                if g == 2 * qs + 1:
                    return qs
                return None

            emit_ablock(0)
            stage_s(0)
            stage_s(1)
            stage_em(0)
            emit_ablock(1)
            # A-blocks inject right after the pvout that frees their psum
            # (steps 1 and 5 are where qs0/qs1 finish). Two proj token tiles
            # per q-slice run promptly; two are deferred to the tail (steps
            # >= n) to keep the PE fed while the last norm chain drains.
            ablock_at = {1: 2, 5: 3}
            norm_q, proj_q = [], []
            # deferred proj tiles land after the last PV so the PE has
            # filler while the final 1/sums chain drains.
            tail_slots = [18, 19, 20, 21, 22, 23]
            pieces = []

            def drain(k=1):
                for _ in range(k):
                    if pieces:
                        pieces.pop(0)()

            for i in range(n + 10):
                while norm_q and norm_q[0][0] <= i:
                    _, dq, pvo, bcast = norm_q.pop(0)
                    emit_norm(dq, pvo, bcast)
                    if dq < NQ - 1:
                        proj_q.append((i + 1, 4 * dq))
                        proj_q.append((i + 2, 4 * dq + 1))
                        for k in (2, 3):
                            proj_q.append((tail_slots.pop(0), 4 * dq + k))
                    else:
                        for k in range(4):
                            proj_q.append((24 + k, 4 * dq + k))
                    proj_q.sort()
                fin = stage_pv(i)
                if fin is not None:
                    pvo, bcast = emit_pvout(fin)
                    # the last slice's norm waits 2 extra steps so the
                    # deferred projection tiles' vector casts aren't queued
                    # behind norm TTs that are waiting on the 1/sums chain.
                    norm_q.append((i + (3 if fin == NQ - 1 else 1), fin, pvo, bcast))
                while proj_q and proj_q[0][0] <= i:
                    slot, tt = proj_q.pop(0)
                    emit_proj_tt(tt, slot >= 18)
                if i in ablock_at:
                    emit_ablock(ablock_at[i], pieces)
                drain()
                stage_s(i + 2)
                drain()
                stage_em(i + 1)
                drain()
            while norm_q:
                _, dq, pvo, bcast = norm_q.pop(0)
                emit_norm(dq, pvo, bcast)
                for k in range(4):
                    proj_q.append((0, 4 * dq + k))
            while proj_q:
                _, tt = proj_q.pop(0)
                emit_proj_tt(tt, True)

    nc.compile()
    return nc


def _get_program():
    if "nc" not in _cache:
        _cache["nc"] = _build()
    return _cache["nc"]


def _prep_in_maps(x, mask, w_qkv, w_proj, head_mask):
    x = np.asarray(x, dtype=np.float32)
    mask = np.asarray(mask, dtype=np.float32)
    w_qkv = np.asarray(w_qkv, dtype=np.float32)
    w_proj = np.asarray(w_proj, dtype=np.float32)
    head_mask = np.asarray(head_mask, dtype=np.float32)

    idx = np.arange(T)
    causal_pen = np.where(idx[:, None] > idx[None, :], NEG, np.float32(0.0))  # [k, q]

    xts, maskts = [], []
    for b in range(B):
        xts.append(np.ascontiguousarray(x[b].T).astype(ml_dtypes.bfloat16))
        em = np.exp(np.ascontiguousarray(mask[b, 0].T) + causal_pen)
        maskts.append(em.astype(ml_dtypes.bfloat16))

    in_maps = []
    for core in range(NCORES):
        b, hg = divmod(core, NCORES // B)
        h0 = hg * HPC
        wq = w_qkv[:, h0 * D : (h0 + HPC) * D] * np.float32(0.125)  # 1/sqrt(D)
        wk = w_qkv[:, C + h0 * D : C + (h0 + HPC) * D]
        wv = w_qkv[:, 2 * C + h0 * D : 2 * C + (h0 + HPC) * D]
        wqkv_c = np.ascontiguousarray(np.concatenate([wq, wk, wv], axis=1)).astype(
            ml_dtypes.bfloat16
        )
        wp = w_proj[h0 * D : (h0 + HPC) * D, :] * np.repeat(head_mask[h0 : h0 + HPC], D)[:, None]
        in_maps.append(
            {
                "xt": xts[b],
                "maskt": maskts[b],
                "wqkv": wqkv_c,
                "wproj": np.ascontiguousarray(wp.astype(ml_dtypes.bfloat16)),
            }
        )
    return in_maps


def run(inputs, trace=False, trace_cores=None):
    nc = _get_program()
    in_maps = _prep_in_maps(**inputs)
    res = run_bass_kernel_spmd(
        nc,
        in_maps,
        list(range(NCORES)),
        trace=trace,
        trace_cores=trace_cores,
    )
    out = np.zeros((B, T, C), dtype=np.float32)
    for core in range(NCORES):
        out[core // (NCORES // B)] += np.asarray(
            res.results[core]["out"], dtype=np.float32
        )
    return out, res


def kernel(x, mask, w_qkv, w_proj, head_mask):
    out, _ = run(dict(x=x, mask=mask, w_qkv=w_qkv, w_proj=w_proj, head_mask=head_mask))
    return out
